# revision 1
# baseline (speedup 1.0000x reference)
"""Trainium2 Bass kernel: Bahdanau (additive) attention with coverage.

Reference computation (per batch element b, data-parallel over B=8 cores):
    enc   = tanh(enc_raw + cov[:,None]*wcov)            [S,H]
    a1    = dec @ Wq + bq                               [T,H]
    a2    = enc @ Wc                                    [S,H]
    scores[t,s] = sum_h v[h] * tanh(a1[t,h] + a2[s,h])  [T,S]
    align = softmax(scores, -1)                         [T,S]
    c     = align @ enc                                 [T,H]
    attn_h = [c, dec] @ Wo + bo                         [T,H]
Outputs: attn_h -> [T,B,H], align -> [T,B,S].

Device strategy: cell-factorized tanh, 7 cells on x = a1 (centers c in
DELTA*{-3..3}, nearest-center quantization, clamped), tau = tanh(x-c),
P = tanh(y+c):
    tanh(x+y) = P + tau(1-P^2) - tau^2 P(1-P^2) + tau^3 P^2(1-P^2) - ...
regrouped in raw powers {P,P^2,P^3,P^4} with the s-independent (softmax-
invariant) components dropped; left coefficients are 4 GLOBAL tiles
(v(1-tau^2), v(-tau+tau^3), v tau^2, -v tau^3) masked per cell, with
tau-order-consistent truncation per cell (degree 3 inner, 2 outer; align
rel err ~4.6e-3 end-to-end vs 2e-2 tolerance).  Engines: ACT does the 7
P-passes (+2 squares), DVE the other squares/P^3/P^4 products, GPSIMD the
26 mask*coeff left products, PE contracts each (left,right) pair over h
into transposed scoresT[s,t] (out free = 64/matmul) plus prologue
matmuls (a1/a2/coverage via eye-matmul PSUM accumulation) and epilogue
(softmax via exp, context and output projections).  All feature tiles
fp16 (DVE 2x/4x perf modes, 1-cyc/row PE); inputs shipped fp16 from the
host in pre-merged [128, k*F] layouts; outputs fp16, upcast on host.
Timeline-simulated exec: ~41.9us/core (baseline exact kernel: 158.8us).
"""

import os

import numpy as np

T, B, S, H = 64, 8, 512, 512
P = 128
KT = H // P   # 4 partition chunks of H
NSC = S // P  # 4 partition chunks of S

C_CELLS = int(os.environ.get("ATTN_CELLS", "7"))
DELTA = float(os.environ.get("ATTN_DELTA", "1.0"))
MAGIC = float(1.5 * 2 ** 23)  # fp32 round-to-nearest-int via add/sub

_BUILT = {}
LAST_RESULT = None


def _emit(nc, tc, ctx, din, dout):
    import concourse.mybir as mybir

    f32 = mybir.dt.float32
    f16 = mybir.dt.float16
    AF = mybir.ActivationFunctionType
    ALU = mybir.AluOpType

    pers = ctx.enter_context(tc.tile_pool(name="pers", bufs=1))
    rt = ctx.enter_context(tc.tile_pool(name="rt", bufs=4))    # right tiles
    lt = ctx.enter_context(tc.tile_pool(name="lt", bufs=3))    # left tiles
    psT = ctx.enter_context(tc.tile_pool(name="psT", bufs=2, space="PSUM"))
    psSm = ctx.enter_context(tc.tile_pool(name="psSm", bufs=2, space="PSUM"))
    psOut = ctx.enter_context(tc.tile_pool(name="psOut", bufs=1, space="PSUM"))

    def ld(name, shape, dt):
        t = pers.tile(shape, dt, tag=name)
        nc.sync.dma_start(out=t[:], in_=din[name][:])
        return t

    # DMA order = need order.
    eye16 = ld("eye16", [P, P], f16)
    encT16 = ld("encT16", [P, KT * S], f16)
    covr16 = ld("cov16", [1, S], f16)
    wcovr16 = ld("wcov16", [1, H], f16)
    wc16 = ld("wc16", [P, KT * H], f16)
    decT16 = ld("decT16", [P, KT * T], f16)
    wq16 = ld("wq16", [P, KT * H], f16)
    bqr16 = ld("bq16", [1, H], f16)
    vrep16 = ld("vrep16", [P, KT * T], f16)
    enc16 = ld("enc16", [P, KT * H], f16)
    wo16 = ld("wo16", [P, 2 * KT * H], f16)
    bor16 = ld("bo16", [1, H], f16)
    eye128 = ld("eye128", [P, P], f32)
    ones16 = pers.tile([1, T], f16, tag="ones16")
    nc.vector.memset(ones16[:], 1.0)
    ones128 = pers.tile([1, P], f16, tag="ones128")
    nc.vector.memset(ones128[:], 1.0)

    # PE p-state warmup: ~3us of dependency-free junk matmuls so the real
    # prologue matmuls run at full clock
    warm = psT.tile([T, T], f32, tag="pt")
    for _ in range(18):
        nc.tensor.matmul(warm[:], ones16[0:1, :], ones16[0:1, :],
                         start=True, stop=True)

    # coverage in [H,S] layout: encT_t = tanh(encT + wcov (x) cov)
    # (adds on GPSIMD to keep DVE free)
    encT_t = pers.tile([P, KT * S], f16, tag="encT_t")
    for i in range(KT):
        op = psT.tile([P, S], f32, tag="pt")
        nc.tensor.matmul(op[:], wcovr16[0:1, i * P:(i + 1) * P], covr16[0:1, :],
                         start=True, stop=False)
        nc.tensor.matmul(op[:], eye16[:], encT16[:, i * S:(i + 1) * S],
                         start=False, stop=True)
        nc.scalar.activation(encT_t[:, i * S:(i + 1) * S], op[:], AF.Tanh)

    # a1T[hout, (k,t)] f32 (feeds cell quantization)
    a1T = pers.tile([P, KT * T], f32, tag="a1T")
    for m in range(KT):
        pm1 = psSm.tile([P, T], f32, tag="ps")
        for k in range(KT):
            nc.tensor.matmul(pm1[:], wq16[:, k * H + m * P:k * H + (m + 1) * P],
                             decT16[:, k * T:(k + 1) * T],
                             start=(k == 0), stop=False)
        nc.tensor.matmul(pm1[:], bqr16[0:1, m * P:(m + 1) * P], ones16[0:1, :],
                         start=False, stop=True)
        nc.vector.tensor_copy(a1T[:, m * T:(m + 1) * T], pm1[:])

    # ---- left-side quantization: cellf = clamp(round(a1/DELTA)), taum = tanh(ctr-a1)
    CH = (C_CELLS - 1) // 2  # centers at DELTA*(-CH..CH)
    if DELTA != 1.0:
        r1 = pers.tile([P, KT * T], f32, tag="r1")
        nc.vector.tensor_scalar(r1[:], a1T[:], float(1.0 / DELTA), None, ALU.mult)
    else:
        r1 = a1T
    cellf = pers.tile([P, KT * T], f32, tag="cellf")
    nc.vector.tensor_scalar(cellf[:], r1[:], MAGIC, MAGIC, ALU.add, ALU.subtract)
    nc.vector.tensor_scalar(cellf[:], cellf[:], float(CH), float(-CH),
                            ALU.min, ALU.max)
    negd = pers.tile([P, KT * T], f32, tag="negd")
    nc.vector.scalar_tensor_tensor(negd[:], cellf[:], float(DELTA), a1T[:],
                                   ALU.mult, ALU.subtract)  # ctr - a1 = -delta
    taum = pers.tile([P, KT * T], f16, tag="taum")
    nc.scalar.activation(taum[:], negd[:], AF.Tanh)
    # global left combos for the raw P-power pairing (softmax-invariant
    # parts of the series dropped):
    #   tanh(x+y) ~ [inv] + (1-tau^2) P + (-tau+tau^3) P^2 + tau^2 P^3 - tau^3 P^4
    # in taum = -tau:  A = v(1-taum^2), B = v(taum-taum^3), C = v taum^2,
    #                  Dg = v taum^3
    vt1 = pers.tile([P, KT * T], f16, tag="vt1")
    nc.vector.tensor_tensor(vt1[:], vrep16[:], taum[:], ALU.mult)
    vt2 = pers.tile([P, KT * T], f16, tag="vt2")
    nc.vector.tensor_tensor(vt2[:], vt1[:], taum[:], ALU.mult)
    uA = pers.tile([P, KT * T], f16, tag="uA")
    nc.vector.tensor_tensor(uA[:], vrep16[:], vt2[:], ALU.subtract)
    uD = pers.tile([P, KT * T], f16, tag="uD")
    nc.vector.tensor_tensor(uD[:], vt2[:], taum[:], ALU.mult)
    uB = pers.tile([P, KT * T], f16, tag="uB")
    nc.vector.tensor_tensor(uB[:], vt1[:], uD[:], ALU.subtract)
    # tau-order-consistent left lists per degree
    upow_by_D = {1: [vrep16, vt1], 2: [uA, vt1, vt2], 3: [uA, uB, vt2, uD]}

    # a2T[hout, (k,s)] merged fp16 tile
    a2T = pers.tile([P, KT * S], f16, tag="a2T")
    for m in range(KT):
        pm2 = psT.tile([P, S], f32, tag="pt")
        for k in range(KT):
            nc.tensor.matmul(pm2[:], wc16[:, k * H + m * P:k * H + (m + 1) * P],
                             encT_t[:, k * S:(k + 1) * S],
                             start=(k == 0), stop=(k == KT - 1))
        nc.scalar.copy(a2T[:, m * S:(m + 1) * S], pm2[:])

    # coverage in [S,H] layout (for the c-matmul): enc_t = tanh(enc + cov (x) wcov)
    enc_t = pers.tile([P, KT * H], f16, tag="enc_t")
    for j in range(NSC):
        op = psT.tile([P, H], f32, tag="pt")
        nc.tensor.matmul(op[:], covr16[0:1, j * P:(j + 1) * P], wcovr16[0:1, :],
                         start=True, stop=False)
        nc.tensor.matmul(op[:], eye16[:], enc16[:, j * H:(j + 1) * H],
                         start=False, stop=True)
        nc.scalar.activation(enc_t[:, j * H:(j + 1) * H], op[:], AF.Tanh)

    # attn_h dec-part (independent of the attention loop): compute early and
    # stage to SBUF so the final merge reads only one PSUM operand
    pa = psOut.tile([T, H], f32, tag="pa")
    for k in range(KT):
        nc.tensor.matmul(pa[:], decT16[:, k * T:(k + 1) * T],
                         wo16[:, (KT + k) * H:(KT + k + 1) * H],
                         start=(k == 0), stop=False)
    nc.tensor.matmul(pa[:], ones16[0:1, :], bor16[0:1, :], start=False,
                     stop=True)
    pa2_sb = pers.tile([T, H], f32, tag="pa2_sb")
    nc.vector.tensor_copy(pa2_sb[:], pa[:])


    # ---- scoresT accumulator; zero the bank once
    scoresT = psOut.tile([P, NSC * T], f32, tag="scT")
    zrow = pers.tile([1, NSC * T], f16, tag="zrow")
    nc.vector.memset(zrow[:], 0.0)
    nc.tensor.matmul(scoresT[:], ones128[0:1, :], zrow[0:1, :],
                     start=True, stop=False)

    # ---- per-cell features + PE contraction
    # series: tanh(x+y) = P + tau*W - tau^2 P W + tau^3 P^2 W   (W = 1-P^2)
    # with taum = -tau and Wm = P^2-1 = -W the pairs are:
    #   (mv, P), (mv*taum, Wm), (mv*taum^2, P*Wm), (mv*taum^3, P^2*Wm)
    n_cells = C_CELLS
    ctrb = pers.tile([P, n_cells], f32, tag="ctrb")
    for ci in range(n_cells):
        nc.vector.memset(ctrb[:, ci:ci + 1], float(DELTA * (ci - CH)))
    # per-cell series degree: outer cells carry ~no probability mass
    DS = ({0: 2, 1: 2, 2: 3, 3: 3, 4: 3, 5: 2, 6: 2} if C_CELLS == 7 else
          {0: 1, 1: 1, 2: 2, 3: 3, 4: 3, 5: 3, 6: 2, 7: 1, 8: 1})
    masks = []
    for ci in range(n_cells):
        mk = pers.tile([P, KT * T], f16, tag=f"mask{ci}")
        nc.vector.tensor_scalar(mk[:], cellf[:], float(ci - CH), None,
                                ALU.is_equal)
        masks.append(mk)
    # Square on DVE for these cells (engine balance); ACT otherwise
    SQ_DVE = {1, 2, 3, 4, 5}
    # emission order center-out so the last cell has the shortest chain
    order = ([3, 2, 4, 1, 5, 0, 6] if C_CELLS == 7 else
             [4, 3, 5, 2, 6, 1, 7, 0, 8])
    for oi, ci in enumerate(order):
        D = DS[ci]
        # left features first: mask*u_k products on GPSIMD (independent of
        # the right-tile chain, so Pool runs ahead)
        ls = []
        for k4 in range(D + 1):
            lk = lt.tile([P, KT * T], f16, tag=f"l{k4}", name=f"l{k4}_{ci}")
            nc.gpsimd.tensor_tensor(lk[:], masks[ci][:], upow_by_D[D][k4][:],
                                    ALU.mult)
            ls.append(lk)

        Pt = rt.tile([P, KT * S], f16, tag="P", name=f"P{ci}")
        nc.scalar.activation(Pt[:], a2T[:], AF.Tanh, bias=ctrb[:, ci:ci + 1])
        P2 = rt.tile([P, KT * S], f16, tag="P2", name=f"P2{ci}")
        if ci in SQ_DVE:
            nc.vector.tensor_tensor(P2[:], Pt[:], Pt[:], ALU.mult)
        else:
            nc.scalar.activation(P2[:], Pt[:], AF.Square)
        rights = [Pt, P2]
        if D >= 2:
            P3 = rt.tile([P, KT * S], f16, tag="P3", name=f"P3{ci}")
            nc.vector.tensor_tensor(P3[:], Pt[:], P2[:], ALU.mult)
            rights.append(P3)
        if D >= 3:
            P4 = rt.tile([P, KT * S], f16, tag="P4", name=f"P4{ci}")
            nc.vector.tensor_tensor(P4[:], P2[:], P2[:], ALU.mult)
            rights.append(P4)

        last_cell = (oi == len(order) - 1)
        # signs: scores += l0*P + l1*P2 + l2*P3 - l3*P4; fold the minus by
        # negating uD once? uD pairs with P4 only, so negate lk on GPSIMD:
        # simpler: uD holds +v*taum^3 = -v*tau^3 and the series term is
        # -tau^3*P4 = +taum^3*P4, so (l3, P4) adds with PLUS sign. Check:
        # series: ... + tau^2 P^3 - tau^3 P^4; l2 = v tau^2 (vt2 = v taum^2
        # = v tau^2, ok), l3 = v taum^3 = -v tau^3 -> l3*P4 = -v tau^3 P4 ok.
        for pi, (Lt, Gt) in enumerate(zip(ls, rights)):
            for k in range(KT):
                for c in range(NSC):
                    stop = (last_cell and pi == D and k == KT - 1
                            and c == NSC - 1)
                    nc.tensor.matmul(
                        scoresT[:, c * T:(c + 1) * T],
                        Gt[:, k * S + c * P:k * S + (c + 1) * P],
                        Lt[:, k * T:(k + 1) * T],
                        start=False, stop=stop)

    # ---- epilogue: transpose, softmax, c, attn_h
    scoresT_sb = pers.tile([P, NSC * T], f32, tag="scT_sb")
    nc.vector.tensor_copy(scoresT_sb[:], scoresT[:])
    if "scdbg" in dout:
        nc.sync.dma_start(out=dout["scdbg"][:], in_=scoresT_sb[:])
    scores = psOut.tile([T, S], f32, tag="out512")
    for c in range(NSC):
        nc.tensor.transpose(scores[:, c * P:(c + 1) * P],
                            scoresT_sb[:, c * T:(c + 1) * T], eye128[:])

    # softmax over s; |scores| is small so exp without max-shift is safe
    expT = pers.tile([P, NSC * T], f16, tag="expT")
    nc.scalar.activation(expT[:], scoresT[:], AF.Exp)
    align_sb = pers.tile([T, S], f16, tag="align_sb")
    sums = pers.tile([T, 1], f32, tag="sums")
    nc.scalar.activation(align_sb[:], scores[:], AF.Exp, accum_out=sums[:])
    recips = pers.tile([T, 1], f32, tag="recips")
    nc.vector.reciprocal(recips[:], sums[:])
    nc.vector.tensor_scalar_mul(align_sb[:], align_sb[:], recips[:])
    nc.sync.dma_start(out=dout["align"][:], in_=align_sb[:])

    # context path straight from the transposed layout: cTu[h,t] =
    # sum_s enc_t[s,h] * exp(scoresT[s,t]); recip applied at the end.
    pc = psOut.tile([P, KT * T], f32, tag="pc")
    nc.tensor.matmul(pc[:], ones128[0:1, :], zrow[0:1, :], start=True, stop=False)
    for j in range(NSC):
        for m in range(KT):
            nc.tensor.matmul(pc[:, m * T:(m + 1) * T],
                             enc_t[:, j * H + m * P:j * H + (m + 1) * P],
                             expT[:, j * T:(j + 1) * T], start=False,
                             stop=(j == NSC - 1 and m == KT - 1))
    cT16 = pers.tile([P, KT * T], f16, tag="cT16")
    nc.vector.tensor_copy(cT16[:], pc[:])

    pa1 = psOut.tile([T, H], f32, tag="out512")
    for k in range(KT):
        nc.tensor.matmul(pa1[:], cT16[:, k * T:(k + 1) * T],
                         wo16[:, k * H:(k + 1) * H],
                         start=(k == 0), stop=(k == KT - 1))
    attn_sb = pers.tile([T, H], f16, tag="attn_sb")
    nc.vector.scalar_tensor_tensor(attn_sb[:], pa1[:], recips[:], pa2_sb[:],
                                   ALU.mult, ALU.add)
    nc.sync.dma_start(out=dout["attn_h"][:], in_=attn_sb[:])


def build(debug_scores=False):
    key = ("cells", debug_scores, C_CELLS, DELTA)
    if key in _BUILT:
        return _BUILT[key]
    from contextlib import ExitStack

    import concourse.bacc as bacc
    import concourse.mybir as mybir
    import concourse.tile as tile

    f32 = mybir.dt.float32
    f16 = mybir.dt.float16
    nc = bacc.Bacc("TRN2", target_bir_lowering=False, debug=False)
    in_specs = [
        ("cov16", [1, S], f16), ("wcov16", [1, H], f16),
        ("encT16", [P, KT * S], f16), ("wc16", [P, KT * H], f16),
        ("decT16", [P, KT * T], f16), ("wq16", [P, KT * H], f16),
        ("bq16", [1, H], f16), ("vrep16", [P, KT * T], f16),
        ("enc16", [P, KT * H], f16), ("wo16", [P, 2 * KT * H], f16),
        ("bo16", [1, H], f16), ("eye128", [P, P], f32),
        ("eye16", [P, P], f16),
    ]
    out_specs = [("attn_h", [T, H], f16), ("align", [T, S], f16)]
    if debug_scores:
        out_specs.append(("scdbg", [P, NSC * T], f32))
    din = {n: nc.declare_dram_parameter(n, s, d, isOutput=False)
           for n, s, d in in_specs}
    dout = {n: nc.declare_dram_parameter(n, s, d, isOutput=True)
            for n, s, d in out_specs}
    with ExitStack() as ctx:
        tc = ctx.enter_context(tile.TileContext(nc))
        _emit(nc, tc, ctx, din, dout)
    nc.compile()
    _BUILT[key] = nc
    return nc


def _merge(x, chunks):
    """[chunks*P, F] -> [P, chunks*F] fp16 (partition-major merge)."""
    cp, F = x.shape
    assert cp == chunks * P
    return np.ascontiguousarray(
        x.reshape(chunks, P, F).transpose(1, 0, 2).reshape(P, chunks * F)
    ).astype(np.float16)


def prep_core_inputs(inputs):
    """Host-side shard: per-core input dicts (core b <- batch element b)."""
    dec = np.asarray(inputs["attn_dec_state"], np.float32)   # [T,B,H]
    encr = np.asarray(inputs["attn_enc_state"], np.float32)  # [S,B,H]
    cov = np.asarray(inputs["attn_coverage"], np.float32)    # [B,S]
    Wq = np.asarray(inputs["Wq"], np.float32)
    Wc = np.asarray(inputs["Wc"], np.float32)
    Wo = np.asarray(inputs["Wo"], np.float32)
    v = np.asarray(inputs["v"], np.float32)
    bq = np.asarray(inputs["bq"], np.float32)[None, :]
    bo = np.asarray(inputs["bo"], np.float32)[None, :]
    wcov = np.asarray(inputs["wcov"], np.float32)[None, :]
    vrep = np.zeros((P, KT * T), np.float32)
    for k in range(KT):
        vrep[:, k * T:(k + 1) * T] = v[k * P:(k + 1) * P][:, None]
    shared = dict(
        wq16=_merge(Wq, KT), wc16=_merge(Wc, KT), wo16=_merge(Wo, 2 * KT),
        vrep16=vrep.astype(np.float16), wcov16=wcov.astype(np.float16),
        bq16=bq.astype(np.float16), bo16=bo.astype(np.float16),
        eye128=np.eye(P, dtype=np.float32),
        eye16=np.eye(P, dtype=np.float16),
    )
    maps = []
    for b in range(B):
        e = np.ascontiguousarray(encr[:, b, :])           # [S,H]
        maps.append(dict(
            decT16=_merge(np.ascontiguousarray(dec[:, b, :].T), KT),
            enc16=_merge(e, NSC),
            encT16=_merge(np.ascontiguousarray(e.T), KT),
            cov16=np.ascontiguousarray(cov[b][None, :]).astype(np.float16),
            **shared,
        ))
    return maps


def kernel(**inputs):
    global LAST_RESULT
    nc = build()
    in_maps = prep_core_inputs(inputs)
    from concourse.bass_utils import run_bass_kernel_spmd

    trace = os.environ.get("ATTN_TRACE", "0") == "1"
    res = run_bass_kernel_spmd(nc, in_maps, list(range(B)), trace=trace)
    LAST_RESULT = res
    attn_h = np.stack([res.results[i]["attn_h"] for i in range(B)],
                      axis=1).astype(np.float32)
    align = np.stack([res.results[i]["align"] for i in range(B)],
                     axis=1).astype(np.float32)
    return attn_h, align



# revision 13
# speedup vs baseline: 1.0714x; 1.0714x over previous
"""Trainium2 Bass kernel: Bahdanau (additive) attention with coverage.

Reference computation (per batch element b, data-parallel over B=8 cores):
    enc   = tanh(enc_raw + cov[:,None]*wcov)            [S,H]
    a1    = dec @ Wq + bq                               [T,H]
    a2    = enc @ Wc                                    [S,H]
    scores[t,s] = sum_h v[h] * tanh(a1[t,h] + a2[s,h])  [T,S]
    align = softmax(scores, -1)                         [T,S]
    c     = align @ enc                                 [T,H]
    attn_h = [c, dec] @ Wo + bo                         [T,H]
Outputs: attn_h -> [T,B,H], align -> [T,B,S].

Device strategy (v2): cell-factorized tanh with 5 cells on x = a1
(DELTA=1.2, centers DELTA*{-2..2}, nearest-center quantization,
clamped), tau = tanh(x-c), P = tanh(y+c):
    tanh(x+y) = P + tau(1-P^2) - tau^2 P(1-P^2) + tau^3 P^2(1-P^2) - ...
regrouped in raw powers {P,P^2,P^3,P^4}, softmax-invariant components
dropped; per-cell degree {2,3,3,3,2} (emulated end-to-end align err
9.7e-3 vs 2e-2 tolerance).  v2 structural changes vs v1:
  - inputs packed into 4 merged DMAs (DMA issue is ~2.2us/copy of
    SP.SEQ+HWDGE+sem-prop overhead in the cost model)
  - a2T accumulated in a 4-bank [128,2048] f32 PSUM tile and kept
    there; the per-cell P-passes tanh straight out of PSUM (kills 4
    PSUM->SBUF copies of v1)
  - encWo = tanh(enc)@Wo_c precontracted on idle PE mid-loop, so the
    epilogue is exp -> (sums/recip | attn matmul) -> normalize, with
    pa_c/pa_d split so the context normalization is a per-partition
    scalar op; align emitted in [s,t] layout (normalized on device)
    and transposed on host
  - 5 cells instead of 7: ACT 5 P-passes, DVE 13-2 products, Pool 18
    mask products, PE 18 pairs
"""

import os

import numpy as np

T, B, S, H = 64, 8, 512, 512
P = 128
KT = H // P   # 4 partition chunks of H
NSC = S // P  # 4 partition chunks of S

N_CELLS = 5
DELTA = 1.2
CH = (N_CELLS - 1) // 2
DEGS = {0: 2, 1: 3, 2: 3, 3: 3, 4: 2}
ORDER = [2, 1, 3, 0, 4]        # center-out emission
SQ_ACT = {0, 4}                # cells whose P^2 runs on ACT (balance)
MAGIC = float(1.5 * 2 ** 23)   # fp32 round-to-nearest-int via add/sub

# input column layout (f16 cols)
GA_N = 2048 + 512 + 512 + 128        # encT | cov | wcov | eye16
GC_N = 2048 + 256 + 256 + 512 + 512  # wq | decT | vrep | bq | bo

_BUILT = {}
LAST_RESULT = None


def _emit(nc, tc, ctx, din, dout):
    import concourse.mybir as mybir

    f32 = mybir.dt.float32
    f16 = mybir.dt.float16
    AF = mybir.ActivationFunctionType
    ALU = mybir.AluOpType

    pers = ctx.enter_context(tc.tile_pool(name="pers", bufs=1))
    rt = ctx.enter_context(tc.tile_pool(name="rt", bufs=6))    # right tiles
    lt = ctx.enter_context(tc.tile_pool(name="lt", bufs=4))    # left tiles
    psBig = ctx.enter_context(tc.tile_pool(name="psBig", bufs=1, space="PSUM"))
    psSc = ctx.enter_context(tc.tile_pool(name="psSc", bufs=1, space="PSUM"))
    psPa = ctx.enter_context(tc.tile_pool(name="psPa", bufs=1, space="PSUM"))
    psEw = ctx.enter_context(tc.tile_pool(name="psEw", bufs=1, space="PSUM"))
    psSm = ctx.enter_context(tc.tile_pool(name="psSm", bufs=1, space="PSUM"))

    # ---- input DMAs (order = need order)
    gA = pers.tile([128, GA_N], f16, tag="gA")
    nc.sync.dma_start(out=gA[:], in_=din["gA"][:])
    gB = pers.tile([128, KT * H], f16, tag="gB")     # wc16
    nc.sync.dma_start(out=gB[:], in_=din["gB"][:])
    gC = pers.tile([128, GC_N], f16, tag="gC")
    nc.sync.dma_start(out=gC[:], in_=din["gC"][:])
    gD = pers.tile([128, 2 * KT * H], f16, tag="gD")  # wo16
    nc.sync.dma_start(out=gD[:], in_=din["gD"][:])

    encT16 = gA[:, 0:2048]
    covr = gA[0:1, 2048:2560]
    wcovr = gA[0:1, 2560:3072]
    eye16 = gA[:, 3072:3200]
    wc16 = gB
    wq16 = gC[:, 0:2048]
    decT16 = gC[:, 2048:2304]
    vrep16 = gC[:, 2304:2560]
    bqr = gC[0:1, 2560:3072]
    bor = gC[0:1, 3072:3584]
    wo16 = gD

    ones16 = pers.tile([1, T], f16, tag="ones16")
    nc.vector.memset(ones16[:], 1.0)
    onesrow = pers.tile([1, P], f16, tag="onesrow")
    nc.vector.memset(onesrow[:], 1.0)
    onescol = pers.tile([P, 1], f16, tag="onescol")
    nc.vector.memset(onescol[:], 1.0)
    ctrb = pers.tile([P, N_CELLS], f32, tag="ctrb")
    for ci in range(N_CELLS):
        nc.vector.memset(ctrb[:, ci:ci + 1], float(DELTA * (ci - CH)))

    # PE p-state warmup: dependency-free junk matmuls so the prologue
    # runs at full clock (cost model: full speed after 3us busy)
    warm = psSm.tile([P, T], f32, tag="pm", name="warm")
    for _ in range(18):
        nc.tensor.matmul(warm[:], onesrow[0:1, :], ones16[0:1, :],
                         start=True, stop=True)

    # ---- encT_t = tanh(encT + wcov (x) cov) in [h,s] layout, via the
    # 4-bank PSUM tile; tanh'd per 512-chunk to start a2T sooner
    bigA = psBig.tile([P, KT * S], f32, tag="big")
    encT_t = pers.tile([P, KT * S], f16, tag="encT_t")
    for k in range(KT):
        sl = bigA[:, k * S:(k + 1) * S]
        nc.tensor.matmul(sl, eye16, encT16[:, k * S:(k + 1) * S],
                         start=True, stop=False)
        nc.tensor.matmul(sl, wcovr[:, k * P:(k + 1) * P], covr[:, 0:S],
                         start=False, stop=True)
        nc.scalar.activation(encT_t[:, k * S:(k + 1) * S], sl, AF.Tanh)

    # ---- a1T[hout,(m,t)] f16 via PSUM m-chunks
    a1T16 = pers.tile([P, KT * T], f16, tag="a1T16")
    for m in range(KT):
        pm = psSm.tile([P, T], f32, tag="pm", name=f"pm{m}")
        for k in range(KT):
            nc.tensor.matmul(pm[:], wq16[:, k * H + m * P:k * H + (m + 1) * P],
                             decT16[:, k * T:(k + 1) * T],
                             start=(k == 0), stop=False)
        nc.tensor.matmul(pm[:], bqr[0:1, m * P:(m + 1) * P], ones16[0:1, :],
                         start=False, stop=True)
        # pm[p,t] currently = a1[t, m*128+p]... no: out = wq_chunk^T decT
        nc.vector.tensor_copy(a1T16[:, m * T:(m + 1) * T], pm[:])

    # ---- quantization chain (f16, tensor_scalar 4x ops)
    q1 = pers.tile([P, KT * T], f32, tag="q1")
    nc.vector.tensor_scalar(q1[:], a1T16[:], float(1.0 / DELTA), None,
                            ALU.mult)
    cellf = pers.tile([P, KT * T], f32, tag="cellf")
    nc.vector.tensor_scalar(cellf[:], q1[:], MAGIC, MAGIC,
                            ALU.add, ALU.subtract)
    nc.vector.tensor_scalar(cellf[:], cellf[:], float(CH), float(-CH),
                            ALU.min, ALU.max)
    negd = pers.tile([P, KT * T], f16, tag="negd")
    nc.vector.scalar_tensor_tensor(negd[:], cellf[:], float(DELTA),
                                   a1T16[:], ALU.mult, ALU.subtract)
    taum = pers.tile([P, KT * T], f16, tag="taum")
    nc.scalar.activation(taum[:], negd[:], AF.Tanh)

    cellf16 = pers.tile([P, KT * T], f16, tag="cellf16")
    nc.vector.tensor_copy(cellf16[:], cellf[:])
    masks = []
    for ci in range(N_CELLS):
        mk = pers.tile([P, KT * T], f16, tag=f"mask{ci}")
        nc.vector.tensor_scalar(mk[:], cellf16[:], float(ci - CH), None,
                                ALU.is_equal)
        masks.append(mk)

    # left basis: A = v(1-taum^2), B = v(taum-taum^3), C = v taum^2,
    # Dg = v taum^3   (taum = -tau)
    vt1 = pers.tile([P, KT * T], f16, tag="vt1")
    nc.vector.tensor_tensor(vt1[:], vrep16, taum[:], ALU.mult)
    vt2 = pers.tile([P, KT * T], f16, tag="vt2")
    nc.vector.tensor_tensor(vt2[:], vt1[:], taum[:], ALU.mult)
    uA = pers.tile([P, KT * T], f16, tag="uA")
    nc.vector.tensor_tensor(uA[:], vrep16, vt2[:], ALU.subtract)
    uD = pers.tile([P, KT * T], f16, tag="uD")
    nc.vector.tensor_tensor(uD[:], vt2[:], taum[:], ALU.mult)
    uB = pers.tile([P, KT * T], f16, tag="uB")
    nc.vector.tensor_tensor(uB[:], vt1[:], uD[:], ALU.subtract)
    upow_by_D = {2: [uA, vt1, vt2], 3: [uA, uB, vt2, uD]}

    # ---- a2T[hout,(m,s)] accumulated into the 4-bank PSUM tile and
    # kept there for the whole cell loop (P-passes read PSUM directly)
    bigB = psBig.tile([P, KT * S], f32, tag="big")
    for m in range(KT):
        sl = bigB[:, m * S:(m + 1) * S]
        for k in range(KT):
            nc.tensor.matmul(sl, wc16[:, k * H + m * P:k * H + (m + 1) * P],
                             encT_t[:, k * S:(k + 1) * S],
                             start=(k == 0), stop=(k == KT - 1))

    # ---- pa_d = dec @ Wo_d + bo (PSUM, read at epilogue)
    pa_d = psPa.tile([T, H], f32, tag="pa_d")
    for k in range(KT):
        nc.tensor.matmul(pa_d[:], decT16[:, k * T:(k + 1) * T],
                         wo16[:, (KT + k) * H:(KT + k + 1) * H],
                         start=(k == 0), stop=False)
    nc.tensor.matmul(pa_d[:], ones16[0:1, :], bor[0:1, :], start=False,
                     stop=True)
    pa_d_sb = pers.tile([T, H], f32, tag="pa_d_sb")
    nc.vector.tensor_copy(pa_d_sb[:], pa_d[:])

    # ---- encWo[s,ho] = tanh(enc) @ Wo_c, on idle PE mid-loop
    encWo16 = pers.tile([P, KT * H], f16, tag="encWo16")
    for j in range(NSC):
        ew = psEw.tile([P, H], f32, tag="ew", name=f"ew{j}")
        for k in range(KT):
            nc.tensor.matmul(ew[:],
                             encT_t[:, k * S + j * P:k * S + (j + 1) * P],
                             wo16[:, k * H:(k + 1) * H],
                             start=(k == 0), stop=(k == KT - 1))
        if j % 2 == 0:
            nc.scalar.copy(encWo16[:, j * H:(j + 1) * H], ew[:])
        else:
            nc.vector.tensor_copy(encWo16[:, j * H:(j + 1) * H], ew[:])

    # ---- cell loop
    scoresT = psSc.tile([P, NSC * T], f32, tag="scT")
    zrow = pers.tile([1, NSC * T], f16, tag="zrow")
    nc.vector.memset(zrow[:], 0.0)
    nc.tensor.matmul(scoresT[:], onesrow[0:1, :], zrow[0:1, :],
                     start=True, stop=False)
    n_pairs = sum(DEGS[ci] + 1 for ci in range(N_CELLS))
    pair_idx = 0
    for oi, ci in enumerate(ORDER):
        D = DEGS[ci]
        ls = []
        for k4 in range(D + 1):
            lk = lt.tile([P, KT * T], f16, tag=f"l{k4}", name=f"l{k4}_{ci}")
            nc.gpsimd.tensor_tensor(lk[:], masks[ci][:], upow_by_D[D][k4][:],
                                    ALU.mult)
            if ci == 2 and k4 == 0 and "l0dbg" in dout:
                nc.sync.dma_start(out=dout["l0dbg"][:], in_=lk[:])
            ls.append(lk)

        Pt = rt.tile([P, KT * S], f16, tag="P", name=f"P{ci}")
        nc.scalar.activation(Pt[:], bigB[:], AF.Tanh, bias=ctrb[:, ci:ci + 1])
        if ci == 2 and "ptdbg" in dout:
            nc.sync.dma_start(out=dout["ptdbg"][:], in_=Pt[:])
        P2 = rt.tile([P, KT * S], f16, tag="P2", name=f"P2{ci}")
        if ci in SQ_ACT:
            nc.scalar.activation(P2[:], Pt[:], AF.Square)
        else:
            nc.vector.tensor_tensor(P2[:], Pt[:], Pt[:], ALU.mult)
        rights = [Pt, P2]
        if D >= 2:
            P3 = rt.tile([P, KT * S], f16, tag="P3", name=f"P3{ci}")
            nc.vector.tensor_tensor(P3[:], Pt[:], P2[:], ALU.mult)
            rights.append(P3)
        if D >= 3:
            P4 = rt.tile([P, KT * S], f16, tag="P4", name=f"P4{ci}")
            nc.vector.tensor_tensor(P4[:], P2[:], P2[:], ALU.mult)
            rights.append(P4)

        for pi, (Lt, Gt) in enumerate(zip(ls, rights)):
            pair_idx += 1
            last_pair = (pair_idx == n_pairs)
            for k in range(KT):
                for c in range(NSC):
                    stop = (last_pair and k == KT - 1 and c == NSC - 1)
                    nc.tensor.matmul(
                        scoresT[:, c * T:(c + 1) * T],
                        Gt[:, k * S + c * P:k * S + (c + 1) * P],
                        Lt[:, k * T:(k + 1) * T],
                        start=False, stop=stop)

    # ---- epilogue
    expT = pers.tile([P, NSC * T], f16, tag="expT")
    nc.scalar.activation(expT[:], scoresT[:], AF.Exp)

    # unnormalized context into pa_c (starts right after exp)
    pa_c_t = psEw.tile([P, H], f32, tag="ew", name="pa_c")
    pa_c = pa_c_t[0:T, :]
    for j in range(NSC):
        nc.tensor.matmul(pa_c, expT[:, j * T:(j + 1) * T],
                         encWo16[:, j * H:(j + 1) * H],
                         start=(j == 0), stop=(j == NSC - 1))

    # sums over s (partition dim) via ones-matmuls; recip; layouts
    sums_t = psSm.tile([P, T], f32, tag="pm", name="sums")
    sums = sums_t[0:1, :]
    for c in range(NSC):
        nc.tensor.matmul(sums, onescol[:], expT[:, c * T:(c + 1) * T],
                         start=(c == 0), stop=(c == NSC - 1))
    recips = pers.tile([1, T], f32, tag="recips")
    nc.vector.reciprocal(recips[:], sums)
    recips16 = pers.tile([1, T], f16, tag="recips16")
    nc.vector.tensor_copy(recips16[:], recips[:])
    # recips as [T,1] column (for the per-partition attn normalize)
    one1 = pers.tile([1, 1], f16, tag="one1")
    nc.vector.memset(one1[:], 1.0)
    rcol_t = psSm.tile([P, T], f32, tag="pm", name="rcol")
    rcol_ps = rcol_t[0:T, 0:1]
    nc.tensor.matmul(rcol_ps, recips16[:], one1[:], start=True, stop=True)
    rcol = pers.tile([T, 1], f32, tag="rcolsb")
    nc.vector.tensor_copy(rcol[:], rcol_ps)
    # recips broadcast [128,T] (for align normalize in [s,t] layout)
    rB_t = psSm.tile([P, T], f32, tag="pm", name="rB")
    nc.tensor.matmul(rB_t[:], onesrow[:], recips16[:], start=True, stop=True)
    rB = pers.tile([P, T], f16, tag="rBsb")
    nc.vector.tensor_copy(rB[:], rB_t[:])

    if "scdbg" in dout:
        sc_sb = pers.tile([P, NSC * T], f32, tag="sc_sb")
        nc.vector.tensor_copy(sc_sb[:], scoresT[:])
        nc.sync.dma_start(out=dout["scdbg"][:], in_=sc_sb[:])
        a1dbg = pers.tile([P, KT * T], f16, tag="a1dbg")
        nc.vector.tensor_copy(a1dbg[:], a1T16[:])
        nc.sync.dma_start(out=dout["a1dbg"][:], in_=a1dbg[:])
        nc.sync.dma_start(out=dout["taudbg"][:], in_=taum[:])
        nc.sync.dma_start(out=dout["cfdbg"][:], in_=cellf16[:])
        nc.sync.dma_start(out=dout["encdbg"][:], in_=encT_t[:])
    alignT = pers.tile([P, NSC * T], f16, tag="alignT")
    for c in range(NSC):
        nc.vector.tensor_tensor(alignT[:, c * T:(c + 1) * T],
                                expT[:, c * T:(c + 1) * T], rB[:], ALU.mult)
    nc.sync.dma_start(out=dout["alignT"][:], in_=alignT[:])

    # attn = pa_c * recip[t] + pa_d
    attn16 = pers.tile([T, H], f16, tag="attn16")
    nc.vector.scalar_tensor_tensor(attn16[:], pa_c, rcol[:], pa_d_sb[:],
                                   ALU.mult, ALU.add)
    nc.sync.dma_start(out=dout["attn_h"][:], in_=attn16[:])


def build(debug=False):
    key = ("v2", debug)
    if key in _BUILT:
        return _BUILT[key]
    from contextlib import ExitStack

    import concourse.bacc as bacc
    import concourse.mybir as mybir
    import concourse.tile as tile

    f16 = mybir.dt.float16
    nc = bacc.Bacc("TRN2", target_bir_lowering=False, debug=False)
    in_specs = [
        ("gA", [128, GA_N], f16), ("gB", [128, KT * H], f16),
        ("gC", [128, GC_N], f16), ("gD", [128, 2 * KT * H], f16),
    ]
    out_specs = [("attn_h", [T, H], f16), ("alignT", [P, NSC * T], f16)]
    if debug:
        out_specs += [("cfdbg", [P, KT * T], f16),
                      ("ptdbg", [P, KT * S], f16),
                      ("l0dbg", [P, KT * T], f16),
                      ("scdbg", [P, NSC * T], mybir.dt.float32),
                      ("a1dbg", [P, KT * T], f16),
                      ("taudbg", [P, KT * T], f16),
                      ("encdbg", [P, KT * S], f16)]
    din = {n: nc.declare_dram_parameter(n, s, d, isOutput=False)
           for n, s, d in in_specs}
    dout = {n: nc.declare_dram_parameter(n, s, d, isOutput=True)
            for n, s, d in out_specs}
    with ExitStack() as ctx:
        tc = ctx.enter_context(tile.TileContext(nc))
        _emit(nc, tc, ctx, din, dout)
    nc.compile()
    _BUILT[key] = nc
    return nc


def _merge(x, chunks):
    """[chunks*P, F] -> [P, chunks*F] fp16 (partition-major merge)."""
    cp, F = x.shape
    assert cp == chunks * P
    return np.ascontiguousarray(
        x.reshape(chunks, P, F).transpose(1, 0, 2).reshape(P, chunks * F)
    ).astype(np.float16)


def prep_core_inputs(inputs):
    """Host-side shard: per-core input dicts (core b <- batch element b)."""
    dec = np.asarray(inputs["attn_dec_state"], np.float32)   # [T,B,H]
    encr = np.asarray(inputs["attn_enc_state"], np.float32)  # [S,B,H]
    cov = np.asarray(inputs["attn_coverage"], np.float32)    # [B,S]
    Wq = np.asarray(inputs["Wq"], np.float32)
    Wc = np.asarray(inputs["Wc"], np.float32)
    Wo = np.asarray(inputs["Wo"], np.float32)
    v = np.asarray(inputs["v"], np.float32)
    bq = np.asarray(inputs["bq"], np.float32)
    bo = np.asarray(inputs["bo"], np.float32)
    wcov = np.asarray(inputs["wcov"], np.float32)
    vrep = np.zeros((P, KT * T), np.float32)
    for k in range(KT):
        vrep[:, k * T:(k + 1) * T] = v[k * P:(k + 1) * P][:, None]

    gB = _merge(Wc, KT)
    wq16 = _merge(Wq, KT)
    gD = _merge(Wo, 2 * KT)
    gC = np.zeros((P, GC_N), np.float16)
    gC[:, 0:2048] = wq16
    gC[:, 2304:2560] = vrep.astype(np.float16)
    gC[0, 2560:3072] = bq.astype(np.float16)
    gC[0, 3072:3584] = bo.astype(np.float16)

    maps = []
    for b in range(B):
        e = np.ascontiguousarray(encr[:, b, :])           # [S,H]
        gA = np.zeros((P, GA_N), np.float16)
        gA[:, 0:2048] = _merge(np.ascontiguousarray(e.T), KT)
        gA[0, 2048:2560] = cov[b].astype(np.float16)
        gA[0, 2560:3072] = wcov.astype(np.float16)
        gA[:, 3072:3200] = np.eye(P, dtype=np.float16)
        gCb = gC.copy()
        gCb[:, 2048:2304] = _merge(
            np.ascontiguousarray(dec[:, b, :].T), KT)
        maps.append(dict(gA=gA, gB=gB, gC=gCb, gD=gD))
    return maps


def kernel(**inputs):
    global LAST_RESULT
    nc = build()
    in_maps = prep_core_inputs(inputs)
    from concourse.bass_utils import run_bass_kernel_spmd

    trace = os.environ.get("ATTN_TRACE", "0") == "1"
    res = run_bass_kernel_spmd(nc, in_maps, list(range(B)), trace=trace)
    LAST_RESULT = res
    attn_h = np.stack([res.results[i]["attn_h"] for i in range(B)],
                      axis=1).astype(np.float32)
    # alignT [s-chunks x t] -> align [t, s]
    align = np.stack(
        [res.results[i]["alignT"].reshape(P, NSC, T).transpose(2, 1, 0)
         .reshape(T, S) for i in range(B)],
        axis=1).astype(np.float32)
    return attn_h, align
